# revision 6
# baseline (speedup 1.0000x reference)
"""Cascaded 5-level stride-2 spatial downsample on Trainium2 (8 NeuronCores).

Math (from the degenerate depthwise convs): down_k = x[:, :, ::2**k, ::2**k]
for k = 1..5 on x of shape (4, 3, 4096, 4096) f32.

Sharding: pure data parallel over H. Core m owns rows [512m, 512(m+1)) of
every (batch, channel) image; 512 is divisible by 32 so every output level
shards cleanly along H. Each core receives its slab flattened to
(12*512, 4096); outputs are concatenated along H on the host.

Device strategy (memory-bound; minimize HBM traffic):
  pass 1: DMA only the EVEN rows of the slab (16KB contiguous chunks,
          stride 32KB), subsample columns on-chip (strided-AP copies),
          write down1 + down2.
  pass 2: DMA rows = 0 (mod 8) of the slab, produce down3/4/5 the same way.
Total HBM traffic per core ~ 63MB read + 34MB written.
"""

import numpy as np

import concourse.bacc as bacc
import concourse.bass as bass
import concourse.mybir as mybir
import concourse.tile as tile
from concourse.bass_utils import run_bass_kernel_spmd

NCORES = 8
NIMG = 12          # 4 batch * 3 channels
H, W = 4096, 4096
SLAB = H // NCORES  # 512 rows per core per image
FLATROWS = NIMG * SLAB  # 6144

F32 = mybir.dt.float32


def build_nc():
    nc = bacc.Bacc()
    x = nc.dram_tensor("x", (FLATROWS, W), F32, kind="ExternalInput")
    d = {
        k: nc.dram_tensor(
            f"d{k}", (FLATROWS >> k, W >> k), F32, kind="ExternalOutput"
        )
        for k in range(1, 6)
    }

    with tile.TileContext(nc) as tc:
        with (
            tc.tile_pool(name="io", bufs=3) as pool,
            tc.tile_pool(name="small", bufs=3) as spool,
        ):
            # ---- pass 1: down1 + down2 from even rows ----
            # iteration t covers flat source rows [512t, 512t+512); partition p
            # holds source rows 512t + 4p + 2j (j in {0,1}).
            for t in range(FLATROWS // 512):
                src = x[512 * t : 512 * (t + 1) : 2, :]  # (256, 4096) even rows
                src3 = src.rearrange("(p j) w -> p j w", j=2)
                xin = pool.tile([128, 2 * W], F32, tag="xin")
                xin3 = xin[:].rearrange("p (j w) -> p j w", j=2)
                nc.sync.dma_start(out=xin3, in_=src3)

                d1t = pool.tile([128, W], F32, tag="d1")
                d1v = d1t[:].rearrange("p (j w) -> p j w", j=2)
                nc.vector.tensor_copy(out=d1v, in_=xin3[:, :, ::2])

                d2t = pool.tile([128, W // 4], F32, tag="d2")
                nc.vector.tensor_copy(out=d2t[:], in_=xin3[:, 0, ::4])

                dst1 = d[1][256 * t : 256 * (t + 1), :].rearrange(
                    "(p j) w -> p j w", j=2
                )
                nc.sync.dma_start(out=dst1, in_=d1v)
                nc.sync.dma_start(out=d[2][128 * t : 128 * (t + 1), :], in_=d2t[:])

            # ---- pass 2: down3/4/5 from rows = 0 (mod 8) ----
            # qualifying rows q = flat_row/8 in [0, 768). group g covers
            # q in [512g, 512g+512) (g=0: 128 partitions, g=1: 64); partition p
            # holds q = 512g + 4p + j (j in 0..3); width in halves h.
            for g, P in ((0, 128), (1, 64)):
                for h in range(2):
                    w0 = (W // 2) * h
                    src = x[4096 * g : 4096 * g + 32 * P : 8, w0 : w0 + W // 2]
                    src3 = src.rearrange("(p j) w -> p j w", j=4)
                    xin = pool.tile([P, 2 * W], F32, tag="xin")
                    xin3 = xin[:, : 4 * (W // 2)].rearrange(
                        "p (j w) -> p j w", j=4
                    )
                    nc.sync.dma_start(out=xin3, in_=src3)

                    d3t = spool.tile([P, 4 * (W // 16)], F32, tag="d3")
                    d3v = d3t[:].rearrange("p (j w) -> p j w", j=4)
                    nc.vector.tensor_copy(out=d3v, in_=xin3[:, :, ::8])

                    d4t = spool.tile([P, 2 * (W // 32)], F32, tag="d4")
                    d4v = d4t[:].rearrange("p (j w) -> p j w", j=2)
                    nc.vector.tensor_copy(out=d4v, in_=xin3[:, ::2, ::16])

                    d5t = spool.tile([P, W // 64], F32, tag="d5")
                    nc.vector.tensor_copy(out=d5t[:], in_=xin3[:, 0, ::32])

                    dst3 = d[3][512 * g : 512 * g + 4 * P, 256 * h : 256 * h + 256]
                    nc.sync.dma_start(
                        out=dst3.rearrange("(p j) w -> p j w", j=4), in_=d3v
                    )
                    dst4 = d[4][256 * g : 256 * g + 2 * P, 128 * h : 128 * h + 128]
                    nc.sync.dma_start(
                        out=dst4.rearrange("(p j) w -> p j w", j=2), in_=d4v
                    )
                    dst5 = d[5][128 * g : 128 * g + P, 64 * h : 64 * h + 64]
                    nc.sync.dma_start(out=dst5, in_=d5t[:])
    nc.finalize()
    return nc


_NC_CACHE = None


def _get_nc():
    global _NC_CACHE
    if _NC_CACHE is None:
        _NC_CACHE = build_nc()
    return _NC_CACHE


def run(x, trace=False):
    """x: full (4, 3, 4096, 4096) f32. Returns (results, tuple_of_5_outputs)."""
    xr = np.asarray(x, dtype=np.float32).reshape(NIMG, H, W)
    in_maps = [
        {
            "x": np.ascontiguousarray(
                xr[:, SLAB * m : SLAB * (m + 1), :]
            ).reshape(FLATROWS, W)
        }
        for m in range(NCORES)
    ]
    nc = _get_nc()
    res = run_bass_kernel_spmd(nc, in_maps, list(range(NCORES)), trace=trace)
    outs = []
    for k in range(1, 6):
        shards = [
            res.results[m][f"d{k}"].reshape(4, 3, SLAB >> k, W >> k)
            for m in range(NCORES)
        ]
        outs.append(np.concatenate(shards, axis=2))
    return res, tuple(outs)


def kernel(x):
    _, outs = run(x)
    return outs


# revision 7
# speedup vs baseline: 1.1909x; 1.1909x over previous
"""Cascaded 5-level stride-2 spatial downsample on Trainium2 (8 NeuronCores).

Math (from the degenerate depthwise convs): down_k = x[:, :, ::2**k, ::2**k]
for k = 1..5 on x of shape (4, 3, 4096, 4096) f32.

Sharding: pure data parallel over H. Core m owns rows [512m, 512(m+1)) of
every (batch, channel) image; 512 is divisible by 32 so every output level
shards cleanly along H. Each core receives its slab flattened to
(12*512, 4096); outputs are concatenated along H on the host.

Device strategy (memory-bound; minimize HBM traffic):
  pass 1: DMA only the EVEN rows of the slab (16KB contiguous chunks,
          stride 32KB) on the SP HWDGE ring, subsample columns on-chip
          (strided-AP copies on DVE), write down1 + down2 on the ACT ring.
  pass 2: re-read down2 (already 4x compacted) from DRAM, produce
          down3/4/5.  RAW ordering on down2 is enforced with explicit
          instruction deps.
Total HBM traffic per core ~ 53.5MB read + 33.5MB written.
"""

import numpy as np

import concourse.bacc as bacc
import concourse.bass as bass
import concourse.mybir as mybir
import concourse.tile as tile
from concourse.tile_rust import add_dep_helper
from concourse.bass_utils import run_bass_kernel_spmd

NCORES = 8
NIMG = 12          # 4 batch * 3 channels
H, W = 4096, 4096
SLAB = H // NCORES  # 512 rows per core per image
FLATROWS = NIMG * SLAB  # 6144

F32 = mybir.dt.float32


def build_nc():
    nc = bacc.Bacc()
    x = nc.dram_tensor("x", (FLATROWS, W), F32, kind="ExternalInput")
    d = {
        k: nc.dram_tensor(
            f"d{k}", (FLATROWS >> k, W >> k), F32, kind="ExternalOutput"
        )
        for k in range(1, 6)
    }

    with tile.TileContext(nc) as tc:
        with (
            tc.tile_pool(name="io", bufs=3) as pool,
            tc.tile_pool(name="small", bufs=2) as spool,
        ):
            d2_writes = []
            # ---- pass 1: down1 + down2 from even rows ----
            # iteration t covers flat source rows [512t, 512t+512); partition p
            # holds source rows 512t + 4p + 2j (j in {0,1}).
            for t in range(FLATROWS // 512):
                src = x[512 * t : 512 * (t + 1) : 2, :]  # (256, 4096) even rows
                src3 = src.rearrange("(p j) w -> p j w", j=2)
                xin = pool.tile([128, 2 * W], F32, tag="xin")
                xin3 = xin[:].rearrange("p (j w) -> p j w", j=2)
                nc.sync.dma_start(out=xin3, in_=src3)

                d1t = pool.tile([128, W], F32, tag="d1")
                d1v = d1t[:].rearrange("p (j w) -> p j w", j=2)
                nc.vector.tensor_copy(out=d1v, in_=xin3[:, :, ::2])

                d2t = pool.tile([128, W // 4], F32, tag="d2")
                nc.vector.tensor_copy(out=d2t[:], in_=xin3[:, 0, ::4])

                dst1 = d[1][256 * t : 256 * (t + 1), :].rearrange(
                    "(p j) w -> p j w", j=2
                )
                nc.scalar.dma_start(out=dst1, in_=d1v)
                w2 = nc.scalar.dma_start(
                    out=d[2][128 * t : 128 * (t + 1), :], in_=d2t[:]
                )
                d2_writes.append(w2.ins)

            # ---- pass 2: down3/4/5 from down2 rows = 0 (mod 2) ----
            # qualifying rows q = d2_row/2 in [0, 768). group g covers
            # q in [512g, 512g+512) (g=0: 128 partitions, g=1: 64);
            # partition p holds q = 512g + 4p + j (j in 0..3), i.e. d2 rows
            # 1024g + 8p + 2j, produced by pass-1 iterations t in [8g, 8g+8).
            for g, P in ((0, 128), (1, 64)):
                src = d[2][1024 * g : 1024 * g + 8 * P : 2, :]  # (4P, 1024)
                src3 = src.rearrange("(p j) w -> p j w", j=4)
                xin = pool.tile([P, W], F32, tag="d1")
                xin3 = xin[:].rearrange("p (j w) -> p j w", j=4)
                rd = nc.sync.dma_start(out=xin3, in_=src3)
                for t in range(8 * g, min(8 * g + 8, len(d2_writes))):
                    add_dep_helper(rd.ins, d2_writes[t], reason="d2 RAW")

                d3t = spool.tile([P, 4 * (W // 8)], F32, tag="d3")
                d3v = d3t[:].rearrange("p (j w) -> p j w", j=4)
                nc.vector.tensor_copy(out=d3v, in_=xin3[:, :, ::2])

                d4t = spool.tile([P, 2 * (W // 16)], F32, tag="d4")
                d4v = d4t[:].rearrange("p (j w) -> p j w", j=2)
                nc.vector.tensor_copy(out=d4v, in_=xin3[:, ::2, ::4])

                d5t = spool.tile([P, W // 32], F32, tag="d5")
                nc.vector.tensor_copy(out=d5t[:], in_=xin3[:, 0, ::8])

                dst3 = d[3][512 * g : 512 * g + 4 * P, :]
                nc.scalar.dma_start(
                    out=dst3.rearrange("(p j) w -> p j w", j=4), in_=d3v
                )
                dst4 = d[4][256 * g : 256 * g + 2 * P, :]
                nc.scalar.dma_start(
                    out=dst4.rearrange("(p j) w -> p j w", j=2), in_=d4v
                )
                nc.scalar.dma_start(
                    out=d[5][128 * g : 128 * g + P, :], in_=d5t[:]
                )
    nc.finalize()
    return nc


_NC_CACHE = None


def _get_nc():
    global _NC_CACHE
    if _NC_CACHE is None:
        _NC_CACHE = build_nc()
    return _NC_CACHE


def run(x, trace=False):
    """x: full (4, 3, 4096, 4096) f32. Returns (results, tuple_of_5_outputs)."""
    xr = np.asarray(x, dtype=np.float32).reshape(NIMG, H, W)
    in_maps = [
        {
            "x": np.ascontiguousarray(
                xr[:, SLAB * m : SLAB * (m + 1), :]
            ).reshape(FLATROWS, W)
        }
        for m in range(NCORES)
    ]
    nc = _get_nc()
    res = run_bass_kernel_spmd(nc, in_maps, list(range(NCORES)), trace=trace)
    outs = []
    for k in range(1, 6):
        shards = [
            res.results[m][f"d{k}"].reshape(4, 3, SLAB >> k, W >> k)
            for m in range(NCORES)
        ]
        outs.append(np.concatenate(shards, axis=2))
    return res, tuple(outs)


def kernel(x):
    _, outs = run(x)
    return outs
